# revision 5
# baseline (speedup 1.0000x reference)
"""Trainium2 Bass kernel for causal multi-head attention (B=2, L=2048, D=2048,
H=16 heads, DH=128), sharded over 8 NeuronCores.

Sharding: core c handles batch b=c//4 and head-group g=c%4 (4 heads = 512
features).

Precision scheme (all-fp16 datapath, fp32 PSUM accumulation):
- q/k activations and Wq/Wk ship as fp16 hi+lo splits (~22-bit effective).
  Q/K projections use 3-term matmuls (hi*hi + lo*hi + hi*lo); the resulting
  qh/kh are re-split into fp16 hi+lo on device, and the attention scores
  S = qh.kh use 3-term matmuls again. This keeps |S| errors ~1e-4 where the
  softmax temperature (x sqrt(128)) would amplify tf32/bf16-level errors to
  percent-level output errors.
- V path, P=softmax(S), attention output, and the final Wo projection run in
  plain fp16 (error contributions ~2-4e-4, no softmax amplification).

Per core:
  1. Q/K/V projections -> QT/KT in (head-dim, seq) hi+lo fp16, V in
     (seq, head-dim) fp16.
  2. Causal attention per head over 128-row q blocks: S (fp32 PSUM), exact
     softmax (ACT exp fused with scale + row-max bias + row-sum accumulation),
     P scaled by 1/l, transposed 128x128 on PE, O^T = V.T @ P^T.
  3. AllGather O^T over the 4-core batch group (fp16, per head-chunk).
  4. out[:, g-slice] = attn_full @ Wo.T[:, g-slice] + bo -> (L, 512) slice.

Host side only reshapes/transposes/splits inputs (layout prep) and
concatenates the 8 output slices.
"""
import sys

sys.path.insert(0, "/opt/trn_rl_repo")

import numpy as np

B, L, D, H = 2, 2048, 2048, 16
DH = D // H          # 128
G = 4                # head-groups (tensor-parallel degree per batch)
HPG = H // G         # heads per group = 4
FPG = HPG * DH       # features per group = 512
P = 128
SCALE = float(DH) ** 0.5
NEG = -1.0e5         # causal mask additive constant (pre-scale)

_COMPILED = None


def _build(variant="main"):
    import concourse.bacc as bacc
    import concourse.tile as tile
    from concourse import mybir
    from contextlib import ExitStack

    F32 = mybir.dt.float32
    F16 = mybir.dt.float16
    AX = mybir.AxisListType
    OP = mybir.AluOpType
    ACTF = mybir.ActivationFunctionType

    nc = bacc.Bacc("TRN2", target_bir_lowering=False, debug=False, num_devices=8)

    # ---- DRAM I/O ----
    xqh = nc.dram_tensor("xqh", [D, L], F16, kind="ExternalInput")
    xql = nc.dram_tensor("xql", [D, L], F16, kind="ExternalInput")
    xkh = nc.dram_tensor("xkh", [D, L], F16, kind="ExternalInput")
    xkl = nc.dram_tensor("xkl", [D, L], F16, kind="ExternalInput")
    xvh = nc.dram_tensor("xvh", [D, L], F16, kind="ExternalInput")
    wqh = nc.dram_tensor("wqh", [D, FPG], F16, kind="ExternalInput")
    wql = nc.dram_tensor("wql", [D, FPG], F16, kind="ExternalInput")
    wkh = nc.dram_tensor("wkh", [D, FPG], F16, kind="ExternalInput")
    wkl = nc.dram_tensor("wkl", [D, FPG], F16, kind="ExternalInput")
    wvh = nc.dram_tensor("wvh", [D, FPG], F16, kind="ExternalInput")
    woT = nc.dram_tensor("woT", [D, FPG], F16, kind="ExternalInput")
    bq = nc.dram_tensor("bq", [FPG, 1], F32, kind="ExternalInput")
    bk = nc.dram_tensor("bk", [FPG, 1], F32, kind="ExternalInput")
    bvb = nc.dram_tensor("bvb", [P, FPG], F32, kind="ExternalInput")
    bob = nc.dram_tensor("bob", [P, FPG], F32, kind="ExternalInput")
    maskd = nc.dram_tensor("maskd", [P, P], F32, kind="ExternalInput")
    identd = nc.dram_tensor("identd", [P, P], F16, kind="ExternalInput")
    out = nc.dram_tensor("out", [L, FPG], F32, kind="ExternalOutput")
    if variant == "timing":
        chain = nc.dram_tensor("chain", [1, 8], F32, kind="ExternalInput")
        dummy = nc.dram_tensor("chaino", [1, 8], F32, kind="ExternalOutput")

    KC = D // P          # 16 contraction chunks
    IB = L // P          # 16 seq blocks of 128
    IPANEL = 256         # projection moving-dim panel
    NPAN = L // IPANEL   # 8

    def drr(t):
        return t.rearrange("(kc p) f -> p kc f", p=P)

    with tile.TileContext(nc) as tc:
        with ExitStack() as ctx:
            consts = ctx.enter_context(tc.tile_pool(name="consts", bufs=1))

            mask_t = consts.tile([P, P], F32)
            nc.sync.dma_start(mask_t[:], maskd[:])
            id_t = consts.tile([P, P], F16)
            nc.sync.dma_start(id_t[:], identd[:])
            bq_t = consts.tile([P, HPG], F32)
            nc.sync.dma_start(bq_t[:], bq.rearrange("(c p) o -> p (c o)", p=P))
            bk_t = consts.tile([P, HPG], F32)
            nc.sync.dma_start(bk_t[:], bk.rearrange("(c p) o -> p (c o)", p=P))
            bvb_t = consts.tile([P, FPG], F32)
            nc.sync.dma_start(bvb_t[:], bvb[:])
            bob_t = consts.tile([P, FPG], F32)
            nc.sync.dma_start(bob_t[:], bob[:])
            if variant == "timing":
                ch_t = consts.tile([1, 8], F32)
                nc.sync.dma_start(ch_t[:], chain[:])
                nc.sync.dma_start(dummy[:], ch_t[:])

            ag_outs = []
            with tc.tile_pool(name="qkv", bufs=1) as qkv:
                qth = qkv.tile([P, HPG, L], F16)   # (d, head, seq) hi
                qtl = qkv.tile([P, HPG, L], F16)   # lo
                kth = qkv.tile([P, HPG, L], F16)
                ktl = qkv.tile([P, HPG, L], F16)
                vt = qkv.tile([P, IB, FPG], F16)   # (seq%128, seq block, feat)

                # ---- phase 1: projections ----
                with tc.tile_pool(name="wpool", bufs=3) as wpool, \
                     tc.tile_pool(name="xpool", bufs=4) as xpool, \
                     tc.tile_pool(name="tpool", bufs=3) as tpool, \
                     tc.tile_pool(name="ppsum", bufs=4, space="PSUM") as ppsum:

                    # Q and K projections -> (feature, seq) hi/lo
                    for (xh_d, xl_d, wh_d, wl_d, bias_t, dh, dl) in (
                        (xqh, xql, wqh, wql, bq_t, qth, qtl),
                        (xkh, xkl, wkh, wkl, bk_t, kth, ktl),
                    ):
                        wh_t = wpool.tile([P, KC, FPG], F16, tag="w")
                        nc.sync.dma_start(wh_t[:], drr(wh_d))
                        wl_t = wpool.tile([P, KC, FPG], F16, tag="w")
                        nc.sync.dma_start(wl_t[:], drr(wl_d))
                        for ip in range(NPAN):
                            isl = slice(ip * IPANEL, (ip + 1) * IPANEL)
                            xh_t = xpool.tile([P, KC, IPANEL], F16, tag="x")
                            nc.sync.dma_start(xh_t[:], drr(xh_d)[:, :, isl])
                            xl_t = xpool.tile([P, KC, IPANEL], F16, tag="x")
                            nc.sync.dma_start(xl_t[:], drr(xl_d)[:, :, isl])
                            for fc in range(HPG):
                                fsl = slice(fc * P, (fc + 1) * P)
                                ps = ppsum.tile([P, IPANEL], F32, tag="pp")
                                for kc in range(KC):
                                    nc.tensor.matmul(
                                        ps[:], wh_t[:, kc, fsl], xh_t[:, kc, :],
                                        start=(kc == 0), stop=False)
                                    nc.tensor.matmul(
                                        ps[:], wh_t[:, kc, fsl], xl_t[:, kc, :],
                                        start=False, stop=False)
                                    nc.tensor.matmul(
                                        ps[:], wl_t[:, kc, fsl], xh_t[:, kc, :],
                                        start=False, stop=(kc == KC - 1))
                                tmp = tpool.tile([P, IPANEL], F32, tag="t")
                                nc.vector.tensor_scalar_add(
                                    tmp[:], ps[:], bias_t[:, fc:fc + 1])
                                nc.vector.tensor_copy(dh[:, fc, isl], tmp[:])
                                nc.vector.tensor_tensor(
                                    dl[:, fc, isl], tmp[:], dh[:, fc, isl],
                                    op=OP.subtract)

                    # V projection -> natural (seq, feature), single term
                    wv_t = wpool.tile([P, KC, FPG], F16, tag="w")
                    nc.sync.dma_start(wv_t[:], drr(wvh))
                    for ip in range(NPAN):
                        isl = slice(ip * IPANEL, (ip + 1) * IPANEL)
                        xv_t = xpool.tile([P, KC, IPANEL], F16, tag="x")
                        nc.sync.dma_start(xv_t[:], drr(xvh)[:, :, isl])
                        for sub in range(IPANEL // P):
                            ib = ip * (IPANEL // P) + sub
                            ps = ppsum.tile([P, FPG], F32, tag="pv")
                            for kc in range(KC):
                                nc.tensor.matmul(
                                    ps[:],
                                    xv_t[:, kc, sub * P:(sub + 1) * P],
                                    wv_t[:, kc, :],
                                    start=(kc == 0), stop=(kc == KC - 1))
                            nc.vector.tensor_tensor(
                                vt[:, ib, :], ps[:], bvb_t[:], op=OP.add)

                # ---- phase 2: attention; AllGather O^T per head-chunk ----
                with tc.tile_pool(name="otpool", bufs=1) as otpool, \
                     tc.tile_pool(name="spsum", bufs=4, space="PSUM") as spsum, \
                     tc.tile_pool(name="tpsum", bufs=2, space="PSUM") as tpsum, \
                     tc.tile_pool(name="opsum", bufs=2, space="PSUM") as opsum, \
                     tc.tile_pool(name="ppool", bufs=2) as ppool, \
                     tc.tile_pool(name="ptpool", bufs=3) as ptpool, \
                     tc.tile_pool(name="stats", bufs=3) as stats, \
                     tc.tile_pool(name="dramio", bufs=1, space="DRAM") as dramio:

                    ot = otpool.tile([P, HPG, L], F16)  # (d, head, seq)

                    for h in range(HPG):
                        for ib in range(IB):
                            nj = (ib + 1) * P
                            nch = (nj + 511) // 512
                            isl = slice(ib * P, (ib + 1) * P)
                            s_ps = []
                            for jc in range(nch):
                                w = min(512, nj - jc * 512)
                                jsl = slice(jc * 512, jc * 512 + w)
                                ps = spsum.tile([P, 512], F32, tag="s")
                                nc.tensor.matmul(
                                    ps[:, :w], qth[:, h, isl], kth[:, h, jsl],
                                    start=True, stop=False)
                                nc.tensor.matmul(
                                    ps[:, :w], qth[:, h, isl], ktl[:, h, jsl],
                                    start=False, stop=False)
                                nc.tensor.matmul(
                                    ps[:, :w], qtl[:, h, isl], kth[:, h, jsl],
                                    start=False, stop=True)
                                s_ps.append((ps, w))
                            # causal mask on the diagonal 128-block (in place)
                            ps, w = s_ps[-1]
                            nc.vector.tensor_tensor(
                                ps[:, w - P:w], ps[:, w - P:w], mask_t[:],
                                op=OP.add)
                            # row-max over chunks
                            mpart = stats.tile([P, 4], F32, tag="mp")
                            for jc, (ps, w) in enumerate(s_ps):
                                nc.vector.reduce_max(
                                    mpart[:, jc:jc + 1], ps[:, :w], axis=AX.X)
                            nmax = stats.tile([P, 1], F32, tag="nm")
                            nc.vector.reduce_max(
                                nmax[:], mpart[:, :nch], axis=AX.X, negate=True)
                            nmax_s = stats.tile([P, 1], F32, tag="nms")
                            nc.vector.tensor_scalar_mul(nmax_s[:], nmax[:], SCALE)
                            # exp(scale*S - scale*max), accumulate row sums
                            p_sb = ppool.tile([P, L], F16, tag="p")
                            lpart = stats.tile([P, 4], F32, tag="lp")
                            for jc, (ps, w) in enumerate(s_ps):
                                nc.scalar.activation(
                                    p_sb[:, jc * 512:jc * 512 + w], ps[:, :w],
                                    ACTF.Exp, bias=nmax_s[:], scale=SCALE,
                                    accum_out=lpart[:, jc:jc + 1])
                            lsum = stats.tile([P, 1], F32, tag="ls")
                            nc.vector.reduce_sum(
                                lsum[:], lpart[:, :nch], axis=AX.X)
                            rinv = stats.tile([P, 1], F32, tag="ri")
                            nc.vector.reciprocal(rinv[:], lsum[:])
                            nc.vector.tensor_scalar_mul(
                                p_sb[:, :nj], p_sb[:, :nj], rinv[:])
                            # transpose P blocks; O^T = V.T @ P^T
                            o_ps = opsum.tile([P, P], F32, tag="o")
                            for jb in range(ib + 1):
                                pt_ps = tpsum.tile([P, P], F16, tag="pt")
                                nc.tensor.transpose(
                                    pt_ps[:], p_sb[:, jb * P:(jb + 1) * P],
                                    id_t[:])
                                pt_sb = ptpool.tile([P, P], F16, tag="ptsb")
                                nc.vector.tensor_copy(pt_sb[:], pt_ps[:])
                                nc.tensor.matmul(
                                    o_ps[:], vt[:, jb, h * P:(h + 1) * P],
                                    pt_sb[:],
                                    start=(jb == 0), stop=(jb == ib))
                            nc.vector.tensor_copy(ot[:, h, isl], o_ps[:])

                        ag_in = dramio.tile([P, L], F16, tag=f"agin{h}")
                        nc.sync.dma_start(ag_in[:], ot[:, h, :])
                        ag_out = dramio.tile([G, P, L], F16, tag=f"agout{h}")
                        if variant == "nocoll":
                            for gg in range(G):
                                nc.sync.dma_start(ag_out[gg], ag_in[:])
                        else:
                            nc.gpsimd.collective_compute(
                                "AllGather", OP.bypass,
                                replica_groups=[[0, 1, 2, 3], [4, 5, 6, 7]],
                                ins=[ag_in[:].opt()], outs=[ag_out[:].opt()])
                        ag_outs.append(ag_out)

            # ---- phase 3: final projection ----
            with tc.tile_pool(name="fpool", bufs=1) as fpool, \
                 tc.tile_pool(name="fopool", bufs=3) as fopool, \
                 tc.tile_pool(name="fpsum", bufs=4, space="PSUM") as fpsum:

                wo_t = fpool.tile([P, KC, FPG], F16)
                nc.sync.dma_start(wo_t[:], drr(woT))
                at_ts = []
                for h in range(HPG):
                    at_t = fpool.tile([P, G, L], F16, tag=f"at{h}")
                    nc.sync.dma_start(
                        at_t[:], ag_outs[h][:].rearrange("g p i -> p g i"))
                    at_ts.append(at_t)

                for ib in range(IB):
                    ps = fpsum.tile([P, FPG], F32, tag="f")
                    for fc in range(KC):
                        g_idx, hc = divmod(fc, HPG)
                        nc.tensor.matmul(
                            ps[:],
                            at_ts[hc][:, g_idx, ib * P:(ib + 1) * P],
                            wo_t[:, fc, :],
                            start=(fc == 0), stop=(fc == KC - 1))
                    o_sb = fopool.tile([P, FPG], F32, tag="fo")
                    nc.vector.tensor_tensor(o_sb[:], ps[:], bob_t[:], op=OP.add)
                    nc.sync.dma_start(out[ib * P:(ib + 1) * P, :], o_sb[:])

    nc.compile()
    return nc


def _split16(x):
    hi = x.astype(np.float16)
    lo = (x - hi.astype(np.float32)).astype(np.float16)
    return hi, lo


def _prepare_in_maps(q, k, v, Wq, bq, Wk, bk, Wv, bv, Wo, bo):
    mask = np.where(
        np.arange(P)[None, :] > np.arange(P)[:, None],
        np.float32(NEG), np.float32(0.0)).astype(np.float32)
    ident = np.eye(P, dtype=np.float16)

    xs = {}
    for b in range(B):
        xs[("q", b)] = _split16(np.ascontiguousarray(q[b].T, dtype=np.float32))
        xs[("k", b)] = _split16(np.ascontiguousarray(k[b].T, dtype=np.float32))
        xs[("v", b)] = np.ascontiguousarray(v[b].T, dtype=np.float32).astype(
            np.float16)

    in_maps = []
    for c in range(8):
        b, g = divmod(c, G)
        F = slice(g * FPG, (g + 1) * FPG)
        wq_h, wq_l = _split16(np.ascontiguousarray(Wq[F, :].T, dtype=np.float32))
        wk_h, wk_l = _split16(np.ascontiguousarray(Wk[F, :].T, dtype=np.float32))
        in_maps.append({
            "xqh": xs[("q", b)][0], "xql": xs[("q", b)][1],
            "xkh": xs[("k", b)][0], "xkl": xs[("k", b)][1],
            "xvh": xs[("v", b)],
            "wqh": wq_h, "wql": wq_l,
            "wkh": wk_h, "wkl": wk_l,
            "wvh": np.ascontiguousarray(Wv[F, :].T).astype(np.float16),
            "woT": np.ascontiguousarray(Wo[F, :].T).astype(np.float16),
            "bq": np.ascontiguousarray(bq[F]).reshape(FPG, 1).astype(np.float32),
            "bk": np.ascontiguousarray(bk[F]).reshape(FPG, 1).astype(np.float32),
            "bvb": np.broadcast_to(bv[F][None, :], (P, FPG)).astype(np.float32),
            "bob": np.broadcast_to(bo[F][None, :], (P, FPG)).astype(np.float32),
            "maskd": mask,
            "identd": ident,
        })
    return in_maps


def kernel(**inputs) -> np.ndarray:
    global _COMPILED
    from concourse.bass_utils import run_bass_kernel_spmd

    if _COMPILED is None:
        _COMPILED = _build()
    nc = _COMPILED

    in_maps = _prepare_in_maps(**inputs)
    res = run_bass_kernel_spmd(nc, in_maps, list(range(8)))

    outp = np.empty((B, L, D), dtype=np.float32)
    for c in range(8):
        b, g = divmod(c, G)
        outp[b, :, g * FPG:(g + 1) * FPG] = res.results[c]["out"]
    return outp


if __name__ == "__main__":
    rng = np.random.default_rng(1)
    ins = {
        "q": rng.standard_normal((B, L, D), dtype=np.float32),
        "k": rng.standard_normal((B, L, D), dtype=np.float32),
        "v": rng.standard_normal((B, L, D), dtype=np.float32),
        "Wq": rng.standard_normal((D, D), dtype=np.float32) * 0.02,
        "bq": rng.standard_normal(D).astype(np.float32) * 0.02,
        "Wk": rng.standard_normal((D, D), dtype=np.float32) * 0.02,
        "bk": rng.standard_normal(D).astype(np.float32) * 0.02,
        "Wv": rng.standard_normal((D, D), dtype=np.float32) * 0.02,
        "bv": rng.standard_normal(D).astype(np.float32) * 0.02,
        "Wo": rng.standard_normal((D, D), dtype=np.float32) * 0.02,
        "bo": rng.standard_normal(D).astype(np.float32) * 0.02,
    }
    o = kernel(**ins)
    print("kernel ran, out shape", o.shape)
